# revision 4
# baseline (speedup 1.0000x reference)
"""Trainium2 Bass kernel for nn_ContrastLoss (contrastive PSD loss).

Math notes (validated against the jax reference on CPU):
  * The band (rfft bins 92..568 of a 4096-point DFT) excludes DC, so the
    mean subtraction in the reference is a no-op for the band PSD.
  * Every _compare() term reduces to rank-1 statistics of the normalized
    PSD matrices (column sums + per-row sum/sumsq); the NxN Gram matrix
    is never materialized.
  * DIF frequency splits: even band bins k=2j need only e=x0+x1 (halves),
    odd bins only d=x0-x1; recursing on the even chain gives sequences
    d[2048] (k odd), eo[1024] (j odd), eeo[512] (j2 odd), eee[512]
    (j2 even).  All folds commute with the sliding crop window.
  * Reversal symmetry: cos(2pi(N-n)k/2N) = +/-cos(2pi nk/2N) (sign by k
    parity), so each cos/sin sum folds to contract length N/2 with data
    (x_n -/+ x_{N-n}).  This halves PE FLOPs again.  eee (even bins) has
    two cos fixed points so it stays unfolded.
  * Everything streams as fp16 (data + DFT tables) with fp32 PSUM
    accumulation; quantization noise averages out over the 2048-crop
    statistics (verified: term rel err ~1.5e-6, loss-term ~4e-5).

Device work per core (1024 crops of the 8192 total): 32 contract slots
of [128, crops] fp16 per crop block -> PE matmuls vs fp16 tables ->
PSUM banks (d-cos|d-sin) and (eo-cos|eo-sin|eeo-cos|eeo-sin|eee) ->
ACT Square (+row-sum accum) -> DVE adds -> band PSD p [128, 477] ->
row sum/sumsq, PE colsum matmul with lhsT = 1/rowsum.  Host combines
the 8 cores' (cs, rowstats) in float64.
"""

import numpy as np

# Problem constants (hardcoded; kernel.py must be self-contained)
B, C, T = 2, 64, 32768
L = 4096
K_CROPS = 32
N_ROWS = C * K_CROPS           # 2048 rows per PSD matrix
N_CORES = 8
ROWS_PER_CORE = N_ROWS * 4 // N_CORES   # 1024
NB = ROWS_PER_CORE // 128      # 8 row blocks per core
F = 477                        # band bins 92..568
FP = F + 1                     # fp32r matmul needs an even moving free dim

# Frequency-bin decomposition (band k = 92..568)
_KBAND = np.arange(92, 569)
K_ODD = _KBAND[_KBAND % 2 == 1]            # 238 bins: d path, N=2048, M=4096
_K2 = _KBAND[_KBAND % 2 == 0] // 2
J_ODD = _K2[_K2 % 2 == 1]                  # 119 bins: eo path, N=1024, M=2048
_J2 = _K2[_K2 % 2 == 0] // 2
J2_ODD = _J2[_J2 % 2 == 1]                 # 60 bins: eeo path, N=512, M=1024
J2_EVEN = _J2[_J2 % 2 == 0]                # 60 bins: eee path (unfolded)

# Contract slot layout: (name, first_slot, n_chunks, bank, col0, ncols)
# bank 0 = [dc 238 | ds 238] (1904B), bank 1 = [eoc 120 | eos 120 |
# o3c 60 | o3s 60 | e3 120] (1920B).  eoc/eos have 119 real bins + 1
# zero-padded col.
SLOTS = [
    ("dc", 0, 8, 0, 0, 238),
    ("ds", 8, 8, 0, 238, 238),
    ("eoc", 16, 4, 1, 0, 120),
    ("eos", 20, 4, 1, 120, 120),
    ("o3c", 24, 2, 1, 240, 60),
    ("o3s", 26, 2, 1, 300, 60),
    ("e3", 28, 4, 1, 360, 120),
]
NSLOT = 32
_PASSES = [[0, 1, 2], [3, 4, 5], [6, 7]]

_NC = None
_W_CACHE = None


def _fold_tables():
    """fp16 DFT tables per path, laid out [128, nch, ncols]
    (row n = chunk*128 + partition)."""
    global _W_CACHE
    if _W_CACHE is not None:
        return _W_CACHE

    def fold_tab(M, ks, ncols_pad):
        # folded cos/sin tables of contract length M/4 (= N/2 with N=M/2)
        half = M // 4
        n = np.arange(half, dtype=np.float64)[:, None]
        ang = 2.0 * np.pi * n * ks[None, :] / M
        Tc = np.cos(ang)
        Ts = np.sin(ang)
        # slot 0 of the sin fold carries x_{N/2} * sin(pi k/2) (k odd)
        Ts[0, :] = np.where(ks % 4 == 1, 1.0, -1.0)
        out = []
        for Tx in (Tc, Ts):
            if ncols_pad > Tx.shape[1]:
                Tx = np.concatenate(
                    [Tx, np.zeros((half, ncols_pad - Tx.shape[1]))], axis=1)
            nch = half // 128
            out.append(np.ascontiguousarray(
                Tx.reshape(nch, 128, ncols_pad).transpose(1, 0, 2)
                .astype(np.float16)))
        return out

    wdc, wds = fold_tab(4096, K_ODD, 238)
    weoc, weos = fold_tab(2048, J_ODD, 120)
    wo3c, wo3s = fold_tab(1024, J2_ODD, 60)
    # eee unfolded: [512, 60 cos | 60 sin]
    n = np.arange(512, dtype=np.float64)[:, None]
    ang = 2.0 * np.pi * n * J2_EVEN[None, :] / 1024.0
    we3 = np.concatenate([np.cos(ang), np.sin(ang)], axis=1)
    we3 = np.ascontiguousarray(
        we3.reshape(4, 128, 120).transpose(1, 0, 2).astype(np.float16))
    _W_CACHE = {"dc": wdc, "ds": wds, "eoc": weoc, "eos": weos,
                "o3c": wo3c, "o3s": wo3s, "e3": we3}
    return _W_CACHE


def _build_module():
    global _NC
    if _NC is not None:
        return _NC
    import concourse.bacc as bacc
    import concourse.bass as bass
    import concourse.tile as tile
    from concourse import mybir

    f32 = mybir.dt.float32
    f32r = mybir.dt.float32r
    f16 = mybir.dt.float16
    AF = mybir.ActivationFunctionType

    nc = bacc.Bacc("TRN2", target_bir_lowering=False, debug=False,
                   num_devices=N_CORES)
    crops_d = [
        nc.dram_tensor(f"crops_p{p}", [128, NSLOT, 128 * len(blks)], f16,
                       kind="ExternalInput")
        for p, blks in enumerate(_PASSES)
    ]
    w_dram = {
        name: nc.dram_tensor(f"w_{name}", [128, nch, ncols], f16,
                             kind="ExternalInput")
        for name, _s0, nch, _bk, _c0, ncols in SLOTS
    }
    out_cs = nc.dram_tensor("out_cs", [1, F], f32, kind="ExternalOutput")
    out_rq = nc.dram_tensor("out_rq", [128, 2 * NB], f32,
                            kind="ExternalOutput")

    # slot -> (name, chunk, bank, col0, ncols, nch)
    slotmap = {}
    for name, s0, nch, bk, c0, ncols in SLOTS:
        for c in range(nch):
            slotmap[s0 + c] = (name, c, bk, c0, ncols, nch)

    with tile.TileContext(nc) as tc:
        with (
            tc.tile_pool(name="wp", bufs=1) as wp,
            tc.tile_pool(name="cp", bufs=2) as cp,
            tc.tile_pool(name="sq", bufs=3) as sqp,
            tc.tile_pool(name="pp", bufs=3) as ppool,
            tc.tile_pool(name="sm", bufs=6) as sm,
            tc.tile_pool(name="outp", bufs=1) as outp,
            tc.tile_pool(name="ps", bufs=7, space=bass.MemorySpace.PSUM) as ps,
            tc.tile_pool(name="pcs", bufs=1, space=bass.MemorySpace.PSUM) as pcs,
        ):
            wt = {name: wp.tile([128, nch, ncols], f16, name=f"wt_{name}")
                  for name, _s0, nch, _bk, _c0, ncols in SLOTS}
            rq_t = outp.tile([128, 2 * NB], f32)
            zero_col = outp.tile([128, 1], f32)
            nc.vector.memset(zero_col, 0.0)
            cs_psum = pcs.tile([1, FP], f32)

            pending = []   # (inv, p_t, blk) colsum matmuls deferred one pass

            def emit_cs(items):
                for c_inv, c_p, c_blk in items:
                    nc.tensor.matmul(cs_psum, c_inv, c_p,
                                     start=(c_blk == 0),
                                     stop=(c_blk == NB - 1))

            def emit_mm(s, j, bA, bB, cpass):
                name, c, bk, c0, ncols, nch = slotmap[s]
                dst = (bA if bk == 0 else bB)[j]
                nc.tensor.matmul(
                    dst[:, c0:c0 + ncols],
                    cpass[:, s, 128 * j:128 * (j + 1)],
                    wt[name][:, c, :],
                    start=(c == 0), stop=(c == nch - 1))

            def emit_post(bA, bB, j, blk):
                sq_a = sqp.tile([128, 476], f32, tag="sqa", name=f"sqa{blk}")
                sq_b = sqp.tile([128, 480], f32, tag="sqb", name=f"sqb{blk}")
                acc_a = sm.tile([128, 1], f32, tag="acca", name=f"acca{blk}")
                acc_b = sm.tile([128, 1], f32, tag="accb", name=f"accb{blk}")
                nc.scalar.activation(out=sq_a, in_=bA[j], func=AF.Square,
                                     accum_out=acc_a)
                nc.scalar.activation(out=sq_b, in_=bB[j], func=AF.Square,
                                     accum_out=acc_b)
                p_t = ppool.tile([128, FP], f32r, tag="p", name=f"p{blk}")
                with nc.allow_low_precision(reason="fp32r is fp32-width"):
                    nc.vector.tensor_add(p_t[:, 0:238], sq_a[:, 0:238],
                                         sq_a[:, 238:476])
                    nc.vector.tensor_add(p_t[:, 238:357], sq_b[:, 0:119],
                                         sq_b[:, 120:239])
                    nc.vector.tensor_add(p_t[:, 357:417], sq_b[:, 240:300],
                                         sq_b[:, 300:360])
                    nc.vector.tensor_add(p_t[:, 417:477], sq_b[:, 360:420],
                                         sq_b[:, 420:480])
                    nc.vector.tensor_copy(p_t[:, F:FP], zero_col)
                rs = rq_t[:, 2 * blk:2 * blk + 1]
                nc.vector.tensor_add(rs, acc_a, acc_b)
                psq = ppool.tile([128, F], f32, tag="psq", name=f"psq{blk}")
                nc.scalar.activation(
                    out=psq, in_=p_t[:, 0:F], func=AF.Square,
                    accum_out=rq_t[:, 2 * blk + 1:2 * blk + 2])
                inv = sm.tile([128, 1], f32r, tag="inv", name=f"inv{blk}")
                with nc.allow_low_precision(reason="fp32r is fp32-width"):
                    nc.vector.reciprocal(inv, rs)
                pending.append((inv, p_t, blk))

            # DMA interleave for pass 0: W tables in consumption order
            w_at = {0: ["dc"], 8: ["ds"], 16: ["eoc", "eos", "o3c", "o3s"],
                    28: ["e3"]}

            for p, blks in enumerate(_PASSES):
                nbp = len(blks)
                ncp = 128 * nbp
                cpass = cp.tile([128, NSLOT, ncp], f16, tag="cp")
                # DMA in consumption order, 4-slot groups
                for s0 in range(0, NSLOT, 4):
                    if p == 0:
                        for wn in w_at.get(s0, []):
                            nc.sync.dma_start(out=wt[wn], in_=w_dram[wn][:])
                    nc.sync.dma_start(out=cpass[:, s0:s0 + 4, :],
                                      in_=crops_d[p][:, s0:s0 + 4, :])

                bA = [ps.tile([128, 476], f32, tag="ps", name=f"bA{p}_{j}")
                      for j in range(nbp)]
                bB = [ps.tile([128, 480], f32, tag="ps", name=f"bB{p}_{j}")
                      for j in range(nbp)]

                if p < len(_PASSES) - 1:
                    # slot-outer: follow DMA arrival order
                    for s in range(NSLOT):
                        for j in range(nbp):
                            emit_mm(s, j, bA, bB, cpass)
                        if s == 12 and pending:
                            emit_cs(pending)
                            pending = []
                    for j, blk in enumerate(blks):
                        emit_post(bA, bB, j, blk)
                else:
                    # final pass: block-outer so earlier blocks' post-chains
                    # overlap later blocks' matmuls (shorter kernel tail)
                    for j, blk in enumerate(blks):
                        for s in range(NSLOT):
                            emit_mm(s, j, bA, bB, cpass)
                        emit_cs(pending)
                        pending = []
                        emit_post(bA, bB, j, blk)

            emit_cs(pending)

            cs_sb = outp.tile([1, F], f32)
            nc.vector.tensor_copy(cs_sb, cs_psum[:, 0:F])
            nc.sync.dma_start(out=out_cs[:], in_=cs_sb)
            nc.sync.dma_start(out=out_rq[:], in_=rq_t)

    nc.compile()
    _NC = nc
    return nc


def _fold_crops(cr):
    """cr [n, 4096] float32 -> packed contract vectors [n, 4096] float32.

    Layout: dc[1024] | ds[1024] | eoc[512] | eos[512] | o3c[256] |
    o3s[256] | eee[512].  Element i of a path sits at slot i//128,
    partition i%128.
    """
    e = cr[:, :2048] + cr[:, 2048:]
    dd = cr[:, :2048] - cr[:, 2048:]
    eo = e[:, :1024] - e[:, 1024:]
    ee = e[:, :1024] + e[:, 1024:]
    eee = ee[:, :512] + ee[:, 512:]
    eeo = ee[:, :512] - ee[:, 512:]
    n = cr.shape[0]
    out = np.empty((n, 4096), np.float32)
    out[:, 0:1024] = dd[:, 0:1024]
    out[:, 1:1024] -= dd[:, 2047:1024:-1]
    out[:, 1024] = dd[:, 1024]
    out[:, 1025:2048] = dd[:, 1:1024] + dd[:, 2047:1024:-1]
    out[:, 2048:2560] = eo[:, 0:512]
    out[:, 2049:2560] -= eo[:, 1023:512:-1]
    out[:, 2560] = eo[:, 512]
    out[:, 2561:3072] = eo[:, 1:512] + eo[:, 1023:512:-1]
    out[:, 3072:3328] = eeo[:, 0:256]
    out[:, 3073:3328] -= eeo[:, 511:256:-1]
    out[:, 3328] = eeo[:, 256]
    out[:, 3329:3584] = eeo[:, 1:256] + eeo[:, 511:256:-1]
    out[:, 3584:4096] = eee
    return out


def _host_prepare(model_output, GT_sig, offsets_st, offsets_t):
    """Build per-core in_maps."""
    from numpy.lib.stride_tricks import sliding_window_view
    wtabs = _fold_tables()
    mats = []   # 4 matrices' packed fold data [2048, 4096] f32
    for b in range(B):
        offs = np.asarray(offsets_st[b], dtype=np.int64).reshape(-1)
        ch_idx = np.repeat(np.arange(C), K_CROPS)
        base = np.asarray(model_output[b], dtype=np.float32)
        win = sliding_window_view(base, L, axis=-1)  # [C, T-L+1, L]
        mats.append(_fold_crops(win[ch_idx, offs]))
    for b in range(B):
        offs = np.asarray(offsets_t[b], dtype=np.int64).reshape(-1)
        sig = np.asarray(GT_sig[b], dtype=np.float32)
        win = sliding_window_view(sig, L)
        mats.append(_fold_crops(win[offs]))

    in_maps = []
    for m in range(4):
        for h in range(2):
            rows = mats[m][h * ROWS_PER_CORE:(h + 1) * ROWS_PER_CORE]
            # [crop, 4096] -> [crop, slot, part] -> [part, slot, crop]
            r = rows.reshape(ROWS_PER_CORE, NSLOT, 128).transpose(2, 1, 0)
            im = {f"w_{k}": v for k, v in wtabs.items()}
            c0 = 0
            for p, blks in enumerate(_PASSES):
                c1 = c0 + 128 * len(blks)
                im[f"crops_p{p}"] = np.ascontiguousarray(
                    r[:, :, c0:c1], dtype=np.float16)
                c0 = c1
            in_maps.append(im)
    return in_maps


def _combine(results, label_flag):
    """results: list of 8 dicts with out_cs [1,F], out_rq [128,2*NB]."""
    cs = np.zeros((4, F), dtype=np.float64)
    ssq = np.zeros(4, dtype=np.float64)
    for m in range(4):
        for h in range(2):
            r = results[2 * m + h]
            cs[m] += np.asarray(r["out_cs"], dtype=np.float64)[0]
            rq = np.asarray(r["out_rq"], dtype=np.float64)
            rs = rq[:, 0::2]
            q = rq[:, 1::2]
            ssq[m] += float(np.sum(q / (rs * rs)))

    N = float(N_ROWS)

    def cmp_excl(a):
        return (2.0 * N * ssq[a] - 2.0 * np.dot(cs[a], cs[a])) / F / (N * (N - 1.0))

    def cmp_full(a, b):
        return (N * ssq[a] + N * ssq[b] - 2.0 * np.dot(cs[a], cs[b])) / F / (N * N)

    lf = np.asarray(label_flag, dtype=np.float64).reshape(-1)
    lf_sum = lf[0] + lf[1]
    denom = 1.0 if lf_sum == 0 else lf_sum
    pos_loss = (cmp_excl(0) + cmp_excl(1)) / 2.0
    neg_loss = -cmp_full(0, 1)
    pos_GT = (lf[0] * cmp_full(0, 2) + lf[1] * cmp_full(1, 3)) / denom
    neg_GT = -(lf[0] * cmp_full(1, 2) + lf[1] * cmp_full(0, 3)) / denom
    if lf_sum == 0:
        pos_GT = 0.0
        neg_GT = 0.0
    loss = pos_loss + neg_loss + pos_GT + neg_GT
    return (np.float32(loss), np.float32(pos_loss), np.float32(neg_loss),
            np.float32(pos_GT), np.float32(neg_GT))


def run(inputs, trace=False):
    """Returns (outputs_tuple, BassKernelResults)."""
    from concourse import bass_utils
    nc = _build_module()
    in_maps = _host_prepare(
        inputs["model_output"], inputs["GT_sig"],
        inputs["offsets_st"], inputs["offsets_t"])
    res = bass_utils.run_bass_kernel_spmd(
        nc, in_maps, core_ids=list(range(N_CORES)), trace=trace)
    outs = _combine(res.results, inputs["label_flag"])
    return outs, res


def kernel(**inputs):
    outs, _ = run(inputs)
    return outs


# revision 10
# speedup vs baseline: 1.0648x; 1.0648x over previous
"""Trainium2 Bass kernel for nn_ContrastLoss (contrastive PSD loss).

Math notes (validated against the jax reference on CPU):
  * The band (rfft bins 92..568 of a 4096-point DFT) excludes DC, so the
    mean subtraction in the reference is a no-op for the band PSD.
  * Every _compare() term reduces to rank-1 statistics of the normalized
    PSD matrices (column sums + per-row sum/sumsq); the NxN Gram matrix
    is never materialized.
  * DIF frequency splits: even band bins k=2j need only e=x0+x1 (halves),
    odd bins only d=x0-x1; recursing on the even chain gives sequences
    d[2048] (k odd), eo[1024] (j odd), eeo[512] (j2 odd), eee[512]
    (j2 even).  All folds commute with the sliding crop window.
  * Reversal symmetry: cos(2pi(N-n)k/2N) = +/-cos(2pi nk/2N) (sign by k
    parity), so each cos/sin sum folds to contract length N/2 with data
    (x_n -/+ x_{N-n}).  This halves PE FLOPs again.  eee (even bins) has
    two cos fixed points so it stays unfolded.
  * Everything streams as fp16 (data + DFT tables) with fp32 PSUM
    accumulation; quantization noise averages out over the 2048-crop
    statistics (verified: term rel err ~1.5e-6, loss-term ~4e-5).

Device work per core (1024 crops of the 8192 total): 32 contract slots
of [128, crops] fp16 per crop block -> PE matmuls vs fp16 tables ->
PSUM banks (d-cos|d-sin) and (eo-cos|eo-sin|eeo-cos|eeo-sin|eee) ->
ACT Square (+row-sum accum) -> DVE adds -> band PSD p [128, 477] ->
row sum/sumsq, PE colsum matmul with lhsT = 1/rowsum.  Host combines
the 8 cores' (cs, rowstats) in float64.
"""

import numpy as np

# Problem constants (hardcoded; kernel.py must be self-contained)
B, C, T = 2, 64, 32768
L = 4096
K_CROPS = 32
N_ROWS = C * K_CROPS           # 2048 rows per PSD matrix
N_CORES = 8
ROWS_PER_CORE = N_ROWS * 4 // N_CORES   # 1024
NB = ROWS_PER_CORE // 128      # 8 row blocks per core
F = 477                        # band bins 92..568
FP = F + 1                     # fp32r matmul needs an even moving free dim

# Frequency-bin decomposition (band k = 92..568)
_KBAND = np.arange(92, 569)
K_ODD = _KBAND[_KBAND % 2 == 1]            # 238 bins: d path, N=2048, M=4096
_K2 = _KBAND[_KBAND % 2 == 0] // 2
J_ODD = _K2[_K2 % 2 == 1]                  # 119 bins: eo path, N=1024, M=2048
_J2 = _K2[_K2 % 2 == 0] // 2
J2_ODD = _J2[_J2 % 2 == 1]                 # 60 bins: eeo path, N=512, M=1024
J2_EVEN = _J2[_J2 % 2 == 0]                # 60 bins: eee path (unfolded)

# Contract slot layout: (name, first_slot, n_chunks, bank, col0, ncols)
# bank 0 = [dc 238 | ds 238] (1904B), bank 1 = [eoc 120 | eos 120 |
# o3c 60 | o3s 60 | e3 120] (1920B).  eoc/eos have 119 real bins + 1
# zero-padded col.
SLOTS = [
    ("dc", 0, 8, 0, 0, 238),
    ("ds", 8, 8, 0, 238, 238),
    ("eoc", 16, 4, 1, 0, 120),
    ("eos", 20, 4, 1, 120, 120),
    ("o3c", 24, 2, 1, 240, 60),
    ("o3s", 26, 2, 1, 300, 60),
    ("e3", 28, 4, 1, 360, 120),
]
NSLOT = 32
_PASSES = [[0, 1, 2], [3, 4, 5], [6, 7]]

_NC = None
_W_CACHE = None


def _fold_tables():
    """fp16 DFT tables per path, laid out [128, nch, ncols]
    (row n = chunk*128 + partition)."""
    global _W_CACHE
    if _W_CACHE is not None:
        return _W_CACHE

    def fold_tab(M, ks, ncols_pad):
        # folded cos/sin tables of contract length M/4 (= N/2 with N=M/2)
        half = M // 4
        n = np.arange(half, dtype=np.float64)[:, None]
        ang = 2.0 * np.pi * n * ks[None, :] / M
        Tc = np.cos(ang)
        Ts = np.sin(ang)
        # slot 0 of the sin fold carries x_{N/2} * sin(pi k/2) (k odd)
        Ts[0, :] = np.where(ks % 4 == 1, 1.0, -1.0)
        out = []
        for Tx in (Tc, Ts):
            if ncols_pad > Tx.shape[1]:
                Tx = np.concatenate(
                    [Tx, np.zeros((half, ncols_pad - Tx.shape[1]))], axis=1)
            nch = half // 128
            out.append(np.ascontiguousarray(
                Tx.reshape(nch, 128, ncols_pad).transpose(1, 0, 2)
                .astype(np.float16)))
        return out

    wdc, wds = fold_tab(4096, K_ODD, 238)
    weoc, weos = fold_tab(2048, J_ODD, 120)
    wo3c, wo3s = fold_tab(1024, J2_ODD, 60)
    # eee unfolded: [512, 60 cos | 60 sin]
    n = np.arange(512, dtype=np.float64)[:, None]
    ang = 2.0 * np.pi * n * J2_EVEN[None, :] / 1024.0
    we3 = np.concatenate([np.cos(ang), np.sin(ang)], axis=1)
    we3 = np.ascontiguousarray(
        we3.reshape(4, 128, 120).transpose(1, 0, 2).astype(np.float16))
    _W_CACHE = {"dc": wdc, "ds": wds, "eoc": weoc, "eos": weos,
                "o3c": wo3c, "o3s": wo3s, "e3": we3}
    return _W_CACHE


def _build_module():
    global _NC
    if _NC is not None:
        return _NC
    import concourse.bacc as bacc
    import concourse.bass as bass
    import concourse.tile as tile
    from concourse import mybir

    f32 = mybir.dt.float32
    f32r = mybir.dt.float32r
    f16 = mybir.dt.float16
    AF = mybir.ActivationFunctionType

    nc = bacc.Bacc("TRN2", target_bir_lowering=False, debug=False,
                   num_devices=N_CORES)
    crops_d = [
        nc.dram_tensor(f"crops_p{p}", [128, NSLOT, 128 * len(blks)], f16,
                       kind="ExternalInput")
        for p, blks in enumerate(_PASSES)
    ]
    # W tables in three DMA-sized groups: dc, ds, and the small paths
    # packed flat (eoc | eos | o3c | o3s | e3) at per-partition offsets.
    w_dc_d = nc.dram_tensor("w_dc", [128, 8, 238], f16, kind="ExternalInput")
    w_ds_d = nc.dram_tensor("w_ds", [128, 8, 238], f16, kind="ExternalInput")
    WSM_OFF = {"eoc": 0, "eos": 480, "o3c": 960, "o3s": 1080, "e3": 1200}
    WSM_LEN = 1680
    w_sm_d = nc.dram_tensor("w_sm", [128, WSM_LEN], f16, kind="ExternalInput")
    out_cs = nc.dram_tensor("out_cs", [1, F], f32, kind="ExternalOutput")
    out_rq = nc.dram_tensor("out_rq", [128, 2 * NB], f32,
                            kind="ExternalOutput")

    # slot -> (name, chunk, bank, col0, ncols, nch)
    slotmap = {}
    for name, s0, nch, bk, c0, ncols in SLOTS:
        for c in range(nch):
            slotmap[s0 + c] = (name, c, bk, c0, ncols, nch)

    with tile.TileContext(nc) as tc:
        with (
            tc.tile_pool(name="wp", bufs=1) as wp,
            tc.tile_pool(name="cp", bufs=2) as cp,
            tc.tile_pool(name="sq", bufs=3) as sqp,
            tc.tile_pool(name="pp", bufs=3) as ppool,
            tc.tile_pool(name="sm", bufs=6) as sm,
            tc.tile_pool(name="outp", bufs=1) as outp,
            tc.tile_pool(name="ps", bufs=7, space=bass.MemorySpace.PSUM) as ps,
            tc.tile_pool(name="pcs", bufs=1, space=bass.MemorySpace.PSUM) as pcs,
        ):
            wt_dc = wp.tile([128, 8, 238], f16)
            wt_ds = wp.tile([128, 8, 238], f16)
            wt_sm = wp.tile([128, WSM_LEN], f16)

            def w_slice(name, c, ncols):
                if name == "dc":
                    return wt_dc[:, c, :]
                if name == "ds":
                    return wt_ds[:, c, :]
                base = WSM_OFF[name]
                return wt_sm[:, base + c * ncols:base + (c + 1) * ncols]
            rq_t = outp.tile([128, 2 * NB], f32)
            zero_col = outp.tile([128, 1], f32)
            nc.vector.memset(zero_col, 0.0)
            cs_psum = pcs.tile([1, FP], f32)

            pending = []   # (inv, p_t, blk) colsum matmuls deferred one pass

            def emit_cs(items):
                for c_inv, c_p, c_blk in items:
                    nc.tensor.matmul(cs_psum, c_inv, c_p,
                                     start=(c_blk == 0),
                                     stop=(c_blk == NB - 1))

            def emit_mm(s, j, bA, bB, cpass):
                name, c, bk, c0, ncols, nch = slotmap[s]
                dst = (bA if bk == 0 else bB)[j]
                nc.tensor.matmul(
                    dst[:, c0:c0 + ncols],
                    cpass[:, s, 128 * j:128 * (j + 1)],
                    w_slice(name, c, ncols),
                    start=(c == 0), stop=(c == nch - 1))

            def emit_post(bA, bB, j, blk):
                sq_a = sqp.tile([128, 476], f32, tag="sqa", name=f"sqa{blk}")
                sq_b = sqp.tile([128, 480], f32, tag="sqb", name=f"sqb{blk}")
                acc_a = sm.tile([128, 1], f32, tag="acca", name=f"acca{blk}")
                acc_b = sm.tile([128, 1], f32, tag="accb", name=f"accb{blk}")
                nc.scalar.activation(out=sq_a, in_=bA[j], func=AF.Square,
                                     accum_out=acc_a)
                nc.scalar.activation(out=sq_b, in_=bB[j], func=AF.Square,
                                     accum_out=acc_b)
                p_t = ppool.tile([128, FP], f32r, tag="p", name=f"p{blk}")
                with nc.allow_low_precision(reason="fp32r is fp32-width"):
                    nc.vector.tensor_add(p_t[:, 0:238], sq_a[:, 0:238],
                                         sq_a[:, 238:476])
                    nc.vector.tensor_add(p_t[:, 238:357], sq_b[:, 0:119],
                                         sq_b[:, 120:239])
                    nc.vector.tensor_add(p_t[:, 357:417], sq_b[:, 240:300],
                                         sq_b[:, 300:360])
                    nc.vector.tensor_add(p_t[:, 417:477], sq_b[:, 360:420],
                                         sq_b[:, 420:480])
                    nc.vector.tensor_copy(p_t[:, F:FP], zero_col)
                rs = rq_t[:, 2 * blk:2 * blk + 1]
                nc.vector.tensor_add(rs, acc_a, acc_b)
                psq = ppool.tile([128, F], f32, tag="psq", name=f"psq{blk}")
                nc.scalar.activation(
                    out=psq, in_=p_t[:, 0:F], func=AF.Square,
                    accum_out=rq_t[:, 2 * blk + 1:2 * blk + 2])
                inv = sm.tile([128, 1], f32r, tag="inv", name=f"inv{blk}")
                with nc.allow_low_precision(reason="fp32r is fp32-width"):
                    nc.vector.reciprocal(inv, rs)
                pending.append((inv, p_t, blk))

            # W DMAs interleaved with crop DMAs in consumption order
            w_at = {0: (wt_dc, w_dc_d), 8: (wt_ds, w_ds_d),
                    16: (wt_sm, w_sm_d)}

            for p, blks in enumerate(_PASSES):
                nbp = len(blks)
                ncp = 128 * nbp
                cpass = cp.tile([128, NSLOT, ncp], f16, tag="cp")
                # DMA in consumption order, 8-slot groups
                for s0 in range(0, NSLOT, 8):
                    if p == 0 and s0 in w_at:
                        w_tile, w_dr = w_at[s0]
                        nc.sync.dma_start(out=w_tile, in_=w_dr[:])
                    nc.sync.dma_start(out=cpass[:, s0:s0 + 8, :],
                                      in_=crops_d[p][:, s0:s0 + 8, :])

                bA = [ps.tile([128, 476], f32, tag="ps", name=f"bA{p}_{j}")
                      for j in range(nbp)]
                bB = [ps.tile([128, 480], f32, tag="ps", name=f"bB{p}_{j}")
                      for j in range(nbp)]

                if p < len(_PASSES) - 1:
                    # slot-outer: follow DMA arrival order
                    for s in range(NSLOT):
                        for j in range(nbp):
                            emit_mm(s, j, bA, bB, cpass)
                        if s == 12 and pending:
                            emit_cs(pending)
                            pending = []
                    for j, blk in enumerate(blks):
                        emit_post(bA, bB, j, blk)
                else:
                    # final pass: block-outer so earlier blocks' post-chains
                    # overlap later blocks' matmuls (shorter kernel tail)
                    for j, blk in enumerate(blks):
                        for s in range(NSLOT):
                            emit_mm(s, j, bA, bB, cpass)
                        emit_cs(pending)
                        pending = []
                        emit_post(bA, bB, j, blk)

            emit_cs(pending)

            cs_sb = outp.tile([1, F], f32)
            nc.vector.tensor_copy(cs_sb, cs_psum[:, 0:F])
            nc.sync.dma_start(out=out_cs[:], in_=cs_sb)
            nc.sync.dma_start(out=out_rq[:], in_=rq_t)

    nc.compile()
    _NC = nc
    return nc


def _fold_crops(cr):
    """cr [n, 4096] float32 -> packed contract vectors [n, 4096] float32.

    Layout: dc[1024] | ds[1024] | eoc[512] | eos[512] | o3c[256] |
    o3s[256] | eee[512].  Element i of a path sits at slot i//128,
    partition i%128.
    """
    e = cr[:, :2048] + cr[:, 2048:]
    dd = cr[:, :2048] - cr[:, 2048:]
    eo = e[:, :1024] - e[:, 1024:]
    ee = e[:, :1024] + e[:, 1024:]
    eee = ee[:, :512] + ee[:, 512:]
    eeo = ee[:, :512] - ee[:, 512:]
    n = cr.shape[0]
    out = np.empty((n, 4096), np.float32)
    out[:, 0:1024] = dd[:, 0:1024]
    out[:, 1:1024] -= dd[:, 2047:1024:-1]
    out[:, 1024] = dd[:, 1024]
    out[:, 1025:2048] = dd[:, 1:1024] + dd[:, 2047:1024:-1]
    out[:, 2048:2560] = eo[:, 0:512]
    out[:, 2049:2560] -= eo[:, 1023:512:-1]
    out[:, 2560] = eo[:, 512]
    out[:, 2561:3072] = eo[:, 1:512] + eo[:, 1023:512:-1]
    out[:, 3072:3328] = eeo[:, 0:256]
    out[:, 3073:3328] -= eeo[:, 511:256:-1]
    out[:, 3328] = eeo[:, 256]
    out[:, 3329:3584] = eeo[:, 1:256] + eeo[:, 511:256:-1]
    out[:, 3584:4096] = eee
    return out


def _host_prepare(model_output, GT_sig, offsets_st, offsets_t):
    """Build per-core in_maps."""
    from numpy.lib.stride_tricks import sliding_window_view
    wtabs = _fold_tables()
    w_sm = np.ascontiguousarray(np.concatenate(
        [wtabs[k].reshape(128, -1) for k in
         ("eoc", "eos", "o3c", "o3s", "e3")], axis=1))
    w_common = {"w_dc": wtabs["dc"], "w_ds": wtabs["ds"], "w_sm": w_sm}
    mats = []   # 4 matrices' packed fold data [2048, 4096] f32
    for b in range(B):
        offs = np.asarray(offsets_st[b], dtype=np.int64).reshape(-1)
        ch_idx = np.repeat(np.arange(C), K_CROPS)
        base = np.asarray(model_output[b], dtype=np.float32)
        win = sliding_window_view(base, L, axis=-1)  # [C, T-L+1, L]
        mats.append(_fold_crops(win[ch_idx, offs]))
    for b in range(B):
        offs = np.asarray(offsets_t[b], dtype=np.int64).reshape(-1)
        sig = np.asarray(GT_sig[b], dtype=np.float32)
        win = sliding_window_view(sig, L)
        mats.append(_fold_crops(win[offs]))

    in_maps = []
    for m in range(4):
        for h in range(2):
            rows = mats[m][h * ROWS_PER_CORE:(h + 1) * ROWS_PER_CORE]
            # [crop, 4096] -> [crop, slot, part] -> [part, slot, crop]
            r = rows.reshape(ROWS_PER_CORE, NSLOT, 128).transpose(2, 1, 0)
            im = dict(w_common)
            c0 = 0
            for p, blks in enumerate(_PASSES):
                c1 = c0 + 128 * len(blks)
                im[f"crops_p{p}"] = np.ascontiguousarray(
                    r[:, :, c0:c1], dtype=np.float16)
                c0 = c1
            in_maps.append(im)
    return in_maps


def _combine(results, label_flag):
    """results: list of 8 dicts with out_cs [1,F], out_rq [128,2*NB]."""
    cs = np.zeros((4, F), dtype=np.float64)
    ssq = np.zeros(4, dtype=np.float64)
    for m in range(4):
        for h in range(2):
            r = results[2 * m + h]
            cs[m] += np.asarray(r["out_cs"], dtype=np.float64)[0]
            rq = np.asarray(r["out_rq"], dtype=np.float64)
            rs = rq[:, 0::2]
            q = rq[:, 1::2]
            ssq[m] += float(np.sum(q / (rs * rs)))

    N = float(N_ROWS)

    def cmp_excl(a):
        return (2.0 * N * ssq[a] - 2.0 * np.dot(cs[a], cs[a])) / F / (N * (N - 1.0))

    def cmp_full(a, b):
        return (N * ssq[a] + N * ssq[b] - 2.0 * np.dot(cs[a], cs[b])) / F / (N * N)

    lf = np.asarray(label_flag, dtype=np.float64).reshape(-1)
    lf_sum = lf[0] + lf[1]
    denom = 1.0 if lf_sum == 0 else lf_sum
    pos_loss = (cmp_excl(0) + cmp_excl(1)) / 2.0
    neg_loss = -cmp_full(0, 1)
    pos_GT = (lf[0] * cmp_full(0, 2) + lf[1] * cmp_full(1, 3)) / denom
    neg_GT = -(lf[0] * cmp_full(1, 2) + lf[1] * cmp_full(0, 3)) / denom
    if lf_sum == 0:
        pos_GT = 0.0
        neg_GT = 0.0
    loss = pos_loss + neg_loss + pos_GT + neg_GT
    return (np.float32(loss), np.float32(pos_loss), np.float32(neg_loss),
            np.float32(pos_GT), np.float32(neg_GT))


def run(inputs, trace=False):
    """Returns (outputs_tuple, BassKernelResults)."""
    from concourse import bass_utils
    nc = _build_module()
    in_maps = _host_prepare(
        inputs["model_output"], inputs["GT_sig"],
        inputs["offsets_st"], inputs["offsets_t"])
    res = bass_utils.run_bass_kernel_spmd(
        nc, in_maps, core_ids=list(range(N_CORES)), trace=trace)
    outs = _combine(res.results, inputs["label_flag"])
    return outs, res


def kernel(**inputs):
    outs, _ = run(inputs)
    return outs
